# revision 1
# baseline (speedup 1.0000x reference)
"""Trainium2 Bass kernel for the DDS sampler problem.

Data-parallel over the batch axis: 8192 samples are split across 8
NeuronCores (1024 each). Each core runs the full 100-step sampler on its
shard; no cross-core communication.

Device data layout is feature-major ([feature, batch] in SBUF) so the
per-step MLP needs no transposes: the host pre-transposes the noise /
x0 / r shards during upload and re-transposes the trajectory output
during the gather.
"""

import numpy as np

# Problem constants (hardcoded per the harness contract).
MB, NS, Z_DIM, R_DIM, HID, NUM_STEPS = 128, 64, 64, 128, 512, 100
B = MB * NS
T_TOTAL = 1.0
ETA = 1.0
DT = T_TOTAL / NUM_STEPS

N_CORES = 8
BS = B // N_CORES          # batch per core
NCH = 2                    # column chunks per core
CW = BS // NCH             # chunk width (512)
KT1 = 128                  # MM1 K-tile split: rows 0:128 = [x; r0:64]
KT2 = Z_DIM + R_DIM - KT1  # rows 128:192 = r[64:128] (t folded into bias)

TRACE = False              # test harness can flip this for profiling
_LAST_RESULT = {}          # test harness introspection (exec_time_ns, trace)

_CACHE = {}


def _schedule(beta_max, beta_min):
    """Cosine-squared schedule constants, in float32 to match the reference."""
    f32 = np.float32
    softplus = lambda x: f32(np.logaddexp(f32(0.0), f32(x)))
    c_start = softplus(beta_max)
    c_end = softplus(beta_min)
    a = f32(c_start - c_end)
    b = f32(np.pi / (2.0 * T_TOTAL))
    c = c_end

    def F(t):
        t = f32(t)
        return f32(a * np.sin(f32(2.0 * b * t)) / (4.0 * b) + a * t / f32(2.0) + c * t)

    steps = np.arange(NUM_STEPS, dtype=np.float32)
    dF = np.array(
        [F((s + 1.0) * DT) - F(s * DT) for s in steps], dtype=np.float32
    )
    alpha = (f32(1.0) - np.exp(f32(-2.0) * dF)).astype(np.float32)
    kappa = ((ETA * (1.0 - np.sqrt(1.0 - alpha))) ** 2 / alpha).astype(np.float32)
    ts = (steps * f32(DT)).astype(np.float32)
    return ts, alpha, kappa


def _build(ts, alpha, kappa):
    """Build + compile the per-core Bass graph. Returns the Bacc object."""
    import concourse.bacc as bacc
    import concourse.mybir as mybir
    import concourse.tile as tile

    f32 = mybir.dt.float32
    Alu = mybir.AluOpType
    Act = mybir.ActivationFunctionType

    nc = bacc.Bacc("TRN2", target_bir_lowering=False, debug=False,
                   num_devices=N_CORES)

    # DRAM parameters (per-core shards; weights replicated).
    noises_d = nc.dram_tensor("noises", [NUM_STEPS, NCH, Z_DIM, CW], f32,
                              kind="ExternalInput")
    x0_d = nc.dram_tensor("x0", [NCH, Z_DIM, CW], f32, kind="ExternalInput")
    r_d = nc.dram_tensor("r", [NCH, R_DIM, CW], f32, kind="ExternalInput")
    w1_d = nc.dram_tensor("w1", [Z_DIM + R_DIM, HID], f32, kind="ExternalInput")
    w2_d = nc.dram_tensor("w2", [128, HID // 128, Z_DIM], f32,
                          kind="ExternalInput")
    b1t_d = nc.dram_tensor("b1t", [128, HID // 128, NUM_STEPS], f32,
                           kind="ExternalInput")
    b2_d = nc.dram_tensor("b2", [Z_DIM, 1], f32, kind="ExternalInput")
    muneg_d = nc.dram_tensor("muneg", [Z_DIM, 1], f32, kind="ExternalInput")

    xt_d = nc.dram_tensor("xt", [NUM_STEPS + 1, NCH, Z_DIM, CW], f32,
                          kind="ExternalOutput")
    lw_d = nc.dram_tensor("lw", [NCH, 1, CW], f32, kind="ExternalOutput")
    nab_d = nc.dram_tensor("nab", [NCH, Z_DIM, CW], f32, kind="ExternalOutput")

    MT = HID // 128  # 4 m-tiles

    with tile.TileContext(nc) as tc:
        with (
            tc.tile_pool(name="const", bufs=1) as cpool,
            tc.tile_pool(name="state", bufs=1) as spool,
            tc.tile_pool(name="noise", bufs=6) as npool,
            tc.tile_pool(name="work", bufs=3) as wpool,
            tc.tile_pool(name="hbuf", bufs=2) as hpool,
            tc.tile_pool(name="hps", bufs=4, space="PSUM") as hps_pool,
            tc.tile_pool(name="sps", bufs=2, space="PSUM") as sps_pool,
        ):
            # --- constants / weights ---
            w1a = cpool.tile([KT1, HID], f32)
            nc.sync.dma_start(out=w1a[:], in_=w1_d[0:KT1, :])
            w1b = cpool.tile([KT2, HID], f32)
            nc.sync.dma_start(out=w1b[:], in_=w1_d[KT1:KT1 + KT2, :])
            w2 = cpool.tile([128, MT, Z_DIM], f32)
            nc.sync.dma_start(out=w2[:], in_=w2_d[:])
            b1t = cpool.tile([128, MT, NUM_STEPS], f32)
            nc.sync.dma_start(out=b1t[:], in_=b1t_d[:])
            b2 = cpool.tile([Z_DIM, 1], f32)
            nc.sync.dma_start(out=b2[:], in_=b2_d[:])
            muneg = cpool.tile([Z_DIM, 1], f32)
            nc.sync.dma_start(out=muneg[:], in_=muneg_d[:])
            ones = cpool.tile([Z_DIM, 1], f32)
            nc.gpsimd.memset(ones[:], 1.0)

            # --- per-chunk state ---
            # inA[parity][c]: [128, CW]; rows 0:64 = x (rewritten every step),
            # rows 64:128 = r[0:64] (constant). inB[c]: [64, CW] = r[64:128].
            inA = [[spool.tile([128, CW], f32, name=f"inA{p}c{c}")
                    for c in range(NCH)] for p in range(2)]
            inB = []
            for c in range(NCH):
                for p in range(2):
                    nc.sync.dma_start(out=inA[p][c][Z_DIM:128, :],
                                      in_=r_d[c, 0:64, :])
                t = spool.tile([KT2, CW], f32, name=f"inB{c}")
                nc.sync.dma_start(out=t[:], in_=r_d[c, 64:128, :])
                inB.append(t)
                nc.sync.dma_start(out=inA[0][c][0:Z_DIM, :], in_=x0_d[c])
                # trajectory slab 0 = x0
                nc.sync.dma_start(out=xt_d[0, c], in_=inA[0][c][0:Z_DIM, :])

            lwacc = [spool.tile([Z_DIM, CW], f32, name=f"lwacc{c}")
                     for c in range(NCH)]
            for c in range(NCH):
                nc.gpsimd.memset(lwacc[c][:], 0.0)

            # --- main loop ---
            for s in range(NUM_STEPS):
                cs = float(1.0 - np.sqrt(1.0 - alpha[s]))
                sa = float(ETA * np.sqrt(alpha[s]))
                m2ka = float(-2.0 * kappa[s])
                m2rka = float(-2.0 * np.sqrt(kappa[s]))
                cur, nxt = inA[s % 2], inA[(s + 1) % 2]
                for c in range(NCH):
                    n_t = npool.tile([Z_DIM, CW], f32, tag="noise")
                    nc.sync.dma_start(out=n_t[:], in_=noises_d[s, c])

                    # MM1: h_pre = [x; r].T-major matmul, K = 128 + 64
                    h = hpool.tile([128, MT, CW], f32, tag=f"h{c}")
                    for m in range(MT):
                        ps = hps_pool.tile([128, CW], f32, tag="hps")
                        nc.tensor.matmul(ps[:], w1a[:, m * 128:(m + 1) * 128],
                                         cur[c][:], start=True, stop=False)
                        nc.tensor.matmul(ps[:], w1b[:, m * 128:(m + 1) * 128],
                                         inB[c][:], start=False, stop=True)
                        # tanh(h_pre + b1 + t_s * W1[192]) fused via bias table
                        nc.scalar.activation(h[:, m, :], ps[:], Act.Tanh,
                                             bias=b1t[:, m, s:s + 1], scale=1.0)

                    # MM2: score = h @ W2 (K = 512 in 4 tiles)
                    sps = sps_pool.tile([Z_DIM, CW], f32, tag="sps")
                    for k in range(MT):
                        nc.tensor.matmul(sps[:], w2[:, k, :], h[:, k, :],
                                         start=(k == 0), stop=(k == MT - 1))
                    sc = wpool.tile([Z_DIM, CW], f32, tag="sc")
                    nc.scalar.activation(sc[:], sps[:], Act.Identity,
                                         bias=b2[:, 0:1], scale=1.0)

                    x = cur[c][0:Z_DIM, :]
                    # log-weight increment: sum_d sc*(-2*ka*sc - 2*sqrt(ka)*n)
                    v = wpool.tile([Z_DIM, CW], f32, tag="v")
                    nc.vector.tensor_scalar(v[:], n_t[:], m2rka, None, Alu.mult)
                    u = wpool.tile([Z_DIM, CW], f32, tag="u")
                    nc.vector.scalar_tensor_tensor(u[:], sc[:], m2ka, v[:],
                                                   Alu.mult, Alu.add)
                    p_t = wpool.tile([Z_DIM, CW], f32, tag="p")
                    nc.vector.tensor_tensor(p_t[:], sc[:], u[:], Alu.mult)
                    nc.vector.tensor_tensor(lwacc[c][:], lwacc[c][:], p_t[:],
                                            Alu.add)

                    # x_new = x + cs*(2*sc - x) + sa*n
                    q = wpool.tile([Z_DIM, CW], f32, tag="q")
                    nc.vector.scalar_tensor_tensor(q[:], sc[:], 2.0, x,
                                                   Alu.mult, Alu.subtract)
                    m_t = wpool.tile([Z_DIM, CW], f32, tag="m")
                    nc.vector.scalar_tensor_tensor(m_t[:], q[:], cs, x,
                                                   Alu.mult, Alu.add)
                    xn = nxt[c][0:Z_DIM, :]
                    nc.vector.scalar_tensor_tensor(xn, n_t[:], sa, m_t[:],
                                                   Alu.mult, Alu.add)
                    nc.sync.dma_start(out=xt_d[s + 1, c], in_=xn)

            # --- epilogue: log_weights and nabla_g ---
            fin = inA[NUM_STEPS % 2]
            for c in range(NCH):
                x = fin[c][0:Z_DIM, :]
                # terminal log-weight: 0.5*(x^2 - (x - mu)^2) summed over d
                xm = wpool.tile([Z_DIM, CW], f32, tag="v")
                nc.scalar.activation(xm[:], x, Act.Square,
                                     bias=muneg[:, 0:1], scale=1.0)
                xsq = wpool.tile([Z_DIM, CW], f32, tag="u")
                nc.scalar.activation(xsq[:], x, Act.Square, bias=0.0, scale=1.0)
                td = wpool.tile([Z_DIM, CW], f32, tag="p")
                nc.vector.tensor_tensor(td[:], xsq[:], xm[:], Alu.subtract)
                tot = wpool.tile([Z_DIM, CW], f32, tag="q")
                nc.vector.scalar_tensor_tensor(tot[:], td[:], 0.5, lwacc[c][:],
                                               Alu.mult, Alu.add)
                lw_ps = sps_pool.tile([1, CW], f32, tag="lwps")
                nc.tensor.matmul(lw_ps[:], ones[:], tot[:],
                                 start=True, stop=True)
                lw_sb = wpool.tile([1, CW], f32, tag="lwsb")
                nc.vector.tensor_copy(lw_sb[:], lw_ps[:])
                nc.sync.dma_start(out=lw_d[c], in_=lw_sb[:])

                # nabla_g = (x - mu) - x  (== -mu up to rounding, as reference)
                d1 = wpool.tile([Z_DIM, CW], f32, tag="m")
                nc.scalar.activation(d1[:], x, Act.Identity,
                                     bias=muneg[:, 0:1], scale=1.0)
                nab = wpool.tile([Z_DIM, CW], f32, tag="sc")
                nc.vector.tensor_tensor(nab[:], d1[:], x, Alu.subtract)
                nc.sync.dma_start(out=nab_d[c], in_=nab[:])

    nc.compile()
    return nc


def kernel(r, noises, x0, W1, b1, W2, b2, beta_max, beta_min, target_mu):
    from concourse.bass_utils import run_bass_kernel_spmd

    r = np.asarray(r, np.float32)
    noises = np.asarray(noises, np.float32)
    x0 = np.asarray(x0, np.float32)
    W1 = np.asarray(W1, np.float32)
    b1 = np.asarray(b1, np.float32)
    W2 = np.asarray(W2, np.float32)
    b2 = np.asarray(b2, np.float32)
    bmax = float(np.asarray(beta_max))
    bmin = float(np.asarray(beta_min))
    mu = np.asarray(target_mu, np.float32)

    ts, alpha, kappa = _schedule(bmax, bmin)

    key = (bmax, bmin)
    if key not in _CACHE:
        _CACHE[key] = _build(ts, alpha, kappa)
    nc = _CACHE[key]

    # Host-side shard prep (pure data movement + the schedule-bias fold).
    r_flat = r.reshape(B, R_DIM)
    # bias table: b1 + t_s * W1[last row] -> [128, 4, S]
    b1t = (b1[None, :] + ts[:, None] * W1[Z_DIM + R_DIM, :]).astype(np.float32)
    b1t = np.ascontiguousarray(
        b1t.reshape(NUM_STEPS, HID // 128, 128).transpose(2, 1, 0))
    w1_in = np.ascontiguousarray(W1[: Z_DIM + R_DIM])
    w2_in = np.ascontiguousarray(
        W2.reshape(HID // 128, 128, Z_DIM).transpose(1, 0, 2))
    b2_in = np.ascontiguousarray(b2[:, None])
    muneg_in = np.ascontiguousarray(-mu[:, None])

    in_maps = []
    for i in range(N_CORES):
        b0, b1i = i * BS, (i + 1) * BS
        nsh = noises[b0:b1i]            # [BS, S, D]
        nT = np.ascontiguousarray(
            nsh.reshape(NCH, CW, NUM_STEPS, Z_DIM).transpose(2, 0, 3, 1))
        x0T = np.ascontiguousarray(
            x0[b0:b1i].reshape(NCH, CW, Z_DIM).transpose(0, 2, 1))
        rT = np.ascontiguousarray(
            r_flat[b0:b1i].reshape(NCH, CW, R_DIM).transpose(0, 2, 1))
        in_maps.append({
            "noises": nT, "x0": x0T, "r": rT,
            "w1": w1_in, "w2": w2_in, "b1t": b1t,
            "b2": b2_in, "muneg": muneg_in,
        })

    res = run_bass_kernel_spmd(nc, in_maps, core_ids=list(range(N_CORES)),
                               trace=TRACE)
    _LAST_RESULT.clear()
    _LAST_RESULT["exec_time_ns"] = res.exec_time_ns
    _LAST_RESULT["trace"] = (res.instructions_and_trace[1]
                             if res.instructions_and_trace else None)

    x_t = np.empty((B, NUM_STEPS + 1, Z_DIM), np.float32)
    log_w = np.empty((B,), np.float32)
    nab = np.empty((B, Z_DIM), np.float32)
    for i in range(N_CORES):
        b0, b1i = i * BS, (i + 1) * BS
        out = res.results[i]
        # [S+1, NCH, D, CW] -> [NCH, CW, S+1, D]
        x_t[b0:b1i] = out["xt"].transpose(1, 3, 0, 2).reshape(
            BS, NUM_STEPS + 1, Z_DIM)
        log_w[b0:b1i] = out["lw"].reshape(BS)
        nab[b0:b1i] = out["nab"].transpose(0, 2, 1).reshape(BS, Z_DIM)
    return x_t, log_w, nab


# revision 7
# speedup vs baseline: 2.6263x; 2.6263x over previous
"""Trainium2 Bass kernel for the DDS sampler problem.

Data-parallel over the batch axis: 8192 samples are split across 8
NeuronCores (1024 each). Each core runs the full 100-step sampler on its
shard; no cross-core communication.

Device data layout is feature-major ([feature, batch] in SBUF) so the
per-step MLP needs no transposes: the host pre-transposes the noise /
x0 / r shards during upload and re-transposes the trajectory output
during the gather.
"""

import numpy as np

# Problem constants (hardcoded per the harness contract).
MB, NS, Z_DIM, R_DIM, HID, NUM_STEPS = 128, 64, 64, 128, 512, 100
B = MB * NS
T_TOTAL = 1.0
ETA = 1.0
DT = T_TOTAL / NUM_STEPS

N_CORES = 8
BS = B // N_CORES          # batch per core
NCH = 2                    # column chunks per core
CW = BS // NCH             # chunk width (512)
KT1 = 128                  # MM1 K-tile split: rows 0:128 = [x; r0:64]
KT2 = Z_DIM + R_DIM - KT1  # rows 128:192 = r[64:128] (t folded into bias)

TRACE = False              # test harness can flip this for profiling
_LAST_RESULT = {}          # test harness introspection (exec_time_ns, trace)

_CACHE = {}


def _schedule(beta_max, beta_min):
    """Cosine-squared schedule constants, in float32 to match the reference."""
    f32 = np.float32
    softplus = lambda x: f32(np.logaddexp(f32(0.0), f32(x)))
    c_start = softplus(beta_max)
    c_end = softplus(beta_min)
    a = f32(c_start - c_end)
    b = f32(np.pi / (2.0 * T_TOTAL))
    c = c_end

    def F(t):
        t = f32(t)
        return f32(a * np.sin(f32(2.0 * b * t)) / (4.0 * b) + a * t / f32(2.0) + c * t)

    steps = np.arange(NUM_STEPS, dtype=np.float32)
    dF = np.array(
        [F((s + 1.0) * DT) - F(s * DT) for s in steps], dtype=np.float32
    )
    alpha = (f32(1.0) - np.exp(f32(-2.0) * dF)).astype(np.float32)
    kappa = ((ETA * (1.0 - np.sqrt(1.0 - alpha))) ** 2 / alpha).astype(np.float32)
    ts = (steps * f32(DT)).astype(np.float32)
    return ts, alpha, kappa


def _build(ts, alpha, kappa):
    """Build + compile the per-core Bass graph. Returns the Bacc object."""
    import concourse.bacc as bacc
    import concourse.mybir as mybir
    import concourse.tile as tile

    f32 = mybir.dt.float32
    f32r = mybir.dt.float32r
    Alu = mybir.AluOpType
    Act = mybir.ActivationFunctionType

    nc = bacc.Bacc("TRN2", target_bir_lowering=False, debug=False,
                   num_devices=N_CORES)

    # DRAM parameters (per-core shards; weights replicated).
    noises_d = nc.dram_tensor("noises", [NUM_STEPS, NCH, Z_DIM, CW], f32,
                              kind="ExternalInput")
    x0_d = nc.dram_tensor("x0", [NCH, Z_DIM, CW], f32r, kind="ExternalInput")
    r_d = nc.dram_tensor("r", [NCH, R_DIM, CW], f32r, kind="ExternalInput")
    w1_d = nc.dram_tensor("w1", [Z_DIM + R_DIM, HID], f32r, kind="ExternalInput")
    w2_d = nc.dram_tensor("w2", [128, HID // 128, Z_DIM], f32r,
                          kind="ExternalInput")
    b1t_d = nc.dram_tensor("b1t", [128, HID // 128, NUM_STEPS], f32,
                           kind="ExternalInput")
    b2t_d = nc.dram_tensor("b2t", [Z_DIM, NUM_STEPS], f32,
                           kind="ExternalInput")
    muneg_d = nc.dram_tensor("muneg", [Z_DIM, 1], f32, kind="ExternalInput")

    xt_d = nc.dram_tensor("xt", [NUM_STEPS + 1, NCH, Z_DIM, CW], f32r,
                          kind="ExternalOutput")
    lw_d = nc.dram_tensor("lw", [NCH, 1, CW], f32, kind="ExternalOutput")
    nab_d = nc.dram_tensor("nab", [NCH, Z_DIM, CW], f32, kind="ExternalOutput")

    MT = HID // 128  # 4 m-tiles

    with tile.TileContext(nc) as tc:
        with (
            tc.tile_pool(name="const", bufs=1) as cpool,
            tc.tile_pool(name="state", bufs=1) as spool,
            tc.tile_pool(name="noise", bufs=6) as npool,
            tc.tile_pool(name="work", bufs=3) as wpool,
            tc.tile_pool(name="hbuf", bufs=2) as hpool,
            tc.tile_pool(name="hps", bufs=4, space="PSUM") as hps_pool,
            tc.tile_pool(name="sps", bufs=2, space="PSUM") as sps_pool,
        ):
            # --- constants / weights ---
            w1a = cpool.tile([KT1, HID], f32r)
            nc.sync.dma_start(out=w1a[:], in_=w1_d[0:KT1, :])
            w1b = cpool.tile([KT2, HID], f32r)
            nc.sync.dma_start(out=w1b[:], in_=w1_d[KT1:KT1 + KT2, :])
            w2 = cpool.tile([128, MT, Z_DIM], f32r)
            nc.sync.dma_start(out=w2[:], in_=w2_d[:])
            b1t = cpool.tile([128, MT, NUM_STEPS], f32)
            nc.sync.dma_start(out=b1t[:], in_=b1t_d[:])
            b2t = cpool.tile([Z_DIM, NUM_STEPS], f32)
            nc.sync.dma_start(out=b2t[:], in_=b2t_d[:])
            muneg = cpool.tile([Z_DIM, 1], f32)
            nc.sync.dma_start(out=muneg[:], in_=muneg_d[:])
            ones = cpool.tile([Z_DIM, 1], f32)
            nc.gpsimd.memset(ones[:], 1.0)

            # --- per-chunk state ---
            # inA[parity][c]: [128, CW]; rows 0:64 = x (rewritten every step),
            # rows 64:128 = r[0:64] (constant). inB[c]: [64, CW] = r[64:128].
            inA = [[spool.tile([128, CW], f32r, name=f"inA{p}c{c}")
                    for c in range(NCH)] for p in range(2)]
            inB = []
            for c in range(NCH):
                for p in range(2):
                    nc.sync.dma_start(out=inA[p][c][Z_DIM:128, :],
                                      in_=r_d[c, 0:64, :])
                t = spool.tile([KT2, CW], f32r, name=f"inB{c}")
                nc.sync.dma_start(out=t[:], in_=r_d[c, 64:128, :])
                inB.append(t)
                nc.sync.dma_start(out=inA[0][c][0:Z_DIM, :], in_=x0_d[c])
                # trajectory slab 0 = x0
                nc.sync.dma_start(out=xt_d[0, c], in_=inA[0][c][0:Z_DIM, :])

            lwacc = [spool.tile([Z_DIM, CW], f32, name=f"lwacc{c}")
                     for c in range(NCH)]
            for c in range(NCH):
                nc.gpsimd.memset(lwacc[c][:], 0.0)

            # --- main loop ---
            for s in range(NUM_STEPS):
                cs = float(1.0 - np.sqrt(1.0 - alpha[s]))
                sa = float(ETA * np.sqrt(alpha[s]))
                cs1 = float(np.float32(1.0) - np.float32(cs))
                twosa = float(2.0 * np.sqrt(alpha[s]))
                nka = float(-1.0 / (2.0 * alpha[s]))
                cur, nxt = inA[s % 2], inA[(s + 1) % 2]
                for c in range(NCH):
                    n_t = npool.tile([Z_DIM, CW], f32, tag="noise")
                    nc.sync.dma_start(out=n_t[:], in_=noises_d[s, c])

                    # MM1: h_pre = [x; r].T-major matmul, K = 128 + 64
                    h = hpool.tile([128, MT, CW], f32r, tag=f"h{c}")
                    for m in range(MT):
                        ps = hps_pool.tile([128, CW], f32, tag="hps")
                        nc.tensor.matmul(ps[:],
                                         w1a[:, m * 128:(m + 1) * 128],
                                         cur[c][:], start=True, stop=False)
                        nc.tensor.matmul(ps[:],
                                         w1b[:, m * 128:(m + 1) * 128],
                                         inB[c][:], start=False, stop=True)
                        # tanh(h_pre + b1 + t_s * W1[192]) fused via bias table
                        nc.scalar.activation(h[:, m, :], ps[:], Act.Tanh,
                                             bias=b1t[:, m, s:s + 1], scale=1.0)

                    # MM2: score = h @ W2 (K = 512 in 4 tiles)
                    sps = sps_pool.tile([Z_DIM, CW], f32, tag="sps")
                    for k in range(MT):
                        nc.tensor.matmul(sps[:], w2[:, k, :], h[:, k, :],
                                         start=(k == 0), stop=(k == MT - 1))
                    # sc2 = 2*cs*(score + b2), via scale/bias fold on ScalarE
                    sc2 = wpool.tile([Z_DIM, CW], f32, tag="sc")
                    nc.scalar.activation(sc2[:], sps[:], Act.Identity,
                                         bias=b2t[:, s:s + 1], scale=2.0 * cs)

                    x = cur[c][0:Z_DIM, :].bitcast(f32)
                    # log-weight increment:
                    #   -2ka*s^2 - 2sqrt(ka)*s*n == -1/(2a) * sc2*(sc2+2*sqrt(a)*n)
                    w_t = wpool.tile([Z_DIM, CW], f32, tag="w")
                    nc.vector.scalar_tensor_tensor(w_t[:], n_t[:], twosa,
                                                   sc2[:], Alu.mult, Alu.add)
                    p_t = wpool.tile([Z_DIM, CW], f32, tag="p")
                    nc.vector.tensor_tensor(p_t[:], sc2[:], w_t[:], Alu.mult)
                    nc.vector.scalar_tensor_tensor(lwacc[c][:], p_t[:], nka,
                                                   lwacc[c][:], Alu.mult,
                                                   Alu.add)

                    # x_new = (1-cs)*x + sc2 + sa*n
                    m_t = wpool.tile([Z_DIM, CW], f32, tag="m")
                    nc.vector.scalar_tensor_tensor(m_t[:], x, cs1, sc2[:],
                                                   Alu.mult, Alu.add)
                    xn = nxt[c][0:Z_DIM, :]  # f32r out: rounds x state
                    nc.vector.scalar_tensor_tensor(xn, n_t[:], sa, m_t[:],
                                                   Alu.mult, Alu.add)
                    nc.sync.dma_start(out=xt_d[s + 1, c], in_=xn)

            # --- epilogue: log_weights and nabla_g ---
            fin = inA[NUM_STEPS % 2]
            for c in range(NCH):
                x = fin[c][0:Z_DIM, :].bitcast(f32)
                # terminal log-weight: 0.5*(x^2 - (x - mu)^2) summed over d
                xm = wpool.tile([Z_DIM, CW], f32, tag="v")
                nc.scalar.activation(xm[:], x, Act.Square,
                                     bias=muneg[:, 0:1], scale=1.0)
                xsq = wpool.tile([Z_DIM, CW], f32, tag="u")
                nc.scalar.activation(xsq[:], x, Act.Square, bias=0.0, scale=1.0)
                td = wpool.tile([Z_DIM, CW], f32, tag="p")
                nc.vector.tensor_tensor(td[:], xsq[:], xm[:], Alu.subtract)
                tot = wpool.tile([Z_DIM, CW], f32, tag="q")
                nc.vector.scalar_tensor_tensor(tot[:], td[:], 0.5, lwacc[c][:],
                                               Alu.mult, Alu.add)
                lw_ps = sps_pool.tile([1, CW], f32, tag="lwps")
                nc.tensor.matmul(lw_ps[:], ones[:], tot[:],
                                 start=True, stop=True)
                lw_sb = wpool.tile([1, CW], f32, tag="lwsb")
                nc.vector.tensor_copy(lw_sb[:], lw_ps[:])
                nc.sync.dma_start(out=lw_d[c], in_=lw_sb[:])

                # nabla_g = (x - mu) - x  (== -mu up to rounding, as reference)
                d1 = wpool.tile([Z_DIM, CW], f32, tag="m")
                nc.scalar.activation(d1[:], x, Act.Identity,
                                     bias=muneg[:, 0:1], scale=1.0)
                nab = wpool.tile([Z_DIM, CW], f32, tag="sc")
                nc.vector.tensor_tensor(nab[:], d1[:], x, Alu.subtract)
                nc.sync.dma_start(out=nab_d[c], in_=nab[:])

    nc.compile()
    return nc


def kernel(r, noises, x0, W1, b1, W2, b2, beta_max, beta_min, target_mu):
    from concourse.bass_utils import run_bass_kernel_spmd

    r = np.asarray(r, np.float32)
    noises = np.asarray(noises, np.float32)
    x0 = np.asarray(x0, np.float32)
    W1 = np.asarray(W1, np.float32)
    b1 = np.asarray(b1, np.float32)
    W2 = np.asarray(W2, np.float32)
    b2 = np.asarray(b2, np.float32)
    bmax = float(np.asarray(beta_max))
    bmin = float(np.asarray(beta_min))
    mu = np.asarray(target_mu, np.float32)

    ts, alpha, kappa = _schedule(bmax, bmin)

    key = (bmax, bmin)
    if key not in _CACHE:
        _CACHE[key] = _build(ts, alpha, kappa)
    nc = _CACHE[key]

    # Host-side shard prep (pure data movement + the schedule-bias fold).
    r_flat = r.reshape(B, R_DIM)
    # bias table: b1 + t_s * W1[last row] -> [128, 4, S]
    b1t = (b1[None, :] + ts[:, None] * W1[Z_DIM + R_DIM, :]).astype(np.float32)
    b1t = np.ascontiguousarray(
        b1t.reshape(NUM_STEPS, HID // 128, 128).transpose(2, 1, 0))
    w1_in = np.ascontiguousarray(W1[: Z_DIM + R_DIM])
    w2_in = np.ascontiguousarray(
        W2.reshape(HID // 128, 128, Z_DIM).transpose(1, 0, 2))
    cs_all = (1.0 - np.sqrt(1.0 - alpha)).astype(np.float32)  # [S]
    b2t_in = np.ascontiguousarray(
        (2.0 * cs_all[None, :] * b2[:, None]).astype(np.float32))
    muneg_in = np.ascontiguousarray(-mu[:, None])

    in_maps = []
    for i in range(N_CORES):
        b0, b1i = i * BS, (i + 1) * BS
        nsh = noises[b0:b1i]            # [BS, S, D]
        nT = np.ascontiguousarray(
            nsh.reshape(NCH, CW, NUM_STEPS, Z_DIM).transpose(2, 0, 3, 1))
        x0T = np.ascontiguousarray(
            x0[b0:b1i].reshape(NCH, CW, Z_DIM).transpose(0, 2, 1))
        rT = np.ascontiguousarray(
            r_flat[b0:b1i].reshape(NCH, CW, R_DIM).transpose(0, 2, 1))
        in_maps.append({
            "noises": nT, "x0": x0T, "r": rT,
            "w1": w1_in, "w2": w2_in, "b1t": b1t,
            "b2t": b2t_in, "muneg": muneg_in,
        })

    res = run_bass_kernel_spmd(nc, in_maps, core_ids=list(range(N_CORES)),
                               trace=TRACE)
    _LAST_RESULT.clear()
    _LAST_RESULT["exec_time_ns"] = res.exec_time_ns
    _LAST_RESULT["trace"] = (res.instructions_and_trace[1]
                             if res.instructions_and_trace else None)

    x_t = np.empty((B, NUM_STEPS + 1, Z_DIM), np.float32)
    log_w = np.empty((B,), np.float32)
    nab = np.empty((B, Z_DIM), np.float32)
    for i in range(N_CORES):
        b0, b1i = i * BS, (i + 1) * BS
        out = res.results[i]
        # [S+1, NCH, D, CW] -> [NCH, CW, S+1, D]
        x_t[b0:b1i] = out["xt"].transpose(1, 3, 0, 2).reshape(
            BS, NUM_STEPS + 1, Z_DIM)
        log_w[b0:b1i] = out["lw"].reshape(BS)
        nab[b0:b1i] = out["nab"].transpose(0, 2, 1).reshape(BS, Z_DIM)
    return x_t, log_w, nab


# revision 9
# speedup vs baseline: 2.6703x; 1.0168x over previous
"""Trainium2 Bass kernel for the DDS sampler problem.

Data-parallel over the batch axis: 8192 samples are split across 8
NeuronCores (1024 each). Each core runs the full 100-step sampler on its
shard; no cross-core communication.

Device data layout is feature-major ([feature, batch] in SBUF) so the
per-step MLP needs no transposes: the host pre-transposes the noise /
x0 / r shards during upload and re-transposes the trajectory output
during the gather.
"""

import numpy as np

# Problem constants (hardcoded per the harness contract).
MB, NS, Z_DIM, R_DIM, HID, NUM_STEPS = 128, 64, 64, 128, 512, 100
B = MB * NS
T_TOTAL = 1.0
ETA = 1.0
DT = T_TOTAL / NUM_STEPS

N_CORES = 8
BS = B // N_CORES          # batch per core
NCH = 2                    # column chunks per core
CW = BS // NCH             # chunk width (512)
KT1 = 128                  # MM1 K-tile split: rows 0:128 = [x; r0:64]
KT2 = Z_DIM + R_DIM - KT1  # rows 128:192 = r[64:128] (t folded into bias)

TRACE = False              # test harness can flip this for profiling
_LAST_RESULT = {}          # test harness introspection (exec_time_ns, trace)

_CACHE = {}


def _schedule(beta_max, beta_min):
    """Cosine-squared schedule constants, in float32 to match the reference."""
    f32 = np.float32
    softplus = lambda x: f32(np.logaddexp(f32(0.0), f32(x)))
    c_start = softplus(beta_max)
    c_end = softplus(beta_min)
    a = f32(c_start - c_end)
    b = f32(np.pi / (2.0 * T_TOTAL))
    c = c_end

    def F(t):
        t = f32(t)
        return f32(a * np.sin(f32(2.0 * b * t)) / (4.0 * b) + a * t / f32(2.0) + c * t)

    steps = np.arange(NUM_STEPS, dtype=np.float32)
    dF = np.array(
        [F((s + 1.0) * DT) - F(s * DT) for s in steps], dtype=np.float32
    )
    alpha = (f32(1.0) - np.exp(f32(-2.0) * dF)).astype(np.float32)
    kappa = ((ETA * (1.0 - np.sqrt(1.0 - alpha))) ** 2 / alpha).astype(np.float32)
    ts = (steps * f32(DT)).astype(np.float32)
    return ts, alpha, kappa


def _build(ts, alpha, kappa):
    """Build + compile the per-core Bass graph. Returns the Bacc object."""
    import concourse.bacc as bacc
    import concourse.mybir as mybir
    import concourse.tile as tile

    f32 = mybir.dt.float32
    f32r = mybir.dt.float32r
    bf16 = mybir.dt.bfloat16
    Alu = mybir.AluOpType
    Act = mybir.ActivationFunctionType

    nc = bacc.Bacc("TRN2", target_bir_lowering=False, debug=False,
                   num_devices=N_CORES)

    # DRAM parameters (per-core shards; weights replicated).
    noises_d = nc.dram_tensor("noises", [NUM_STEPS, NCH, Z_DIM, CW], f32,
                              kind="ExternalInput")
    x0_d = nc.dram_tensor("x0", [NCH, Z_DIM, CW], f32, kind="ExternalInput")
    r_d = nc.dram_tensor("r", [NCH, R_DIM, CW], bf16, kind="ExternalInput")
    w1_d = nc.dram_tensor("w1", [Z_DIM + R_DIM, HID], bf16, kind="ExternalInput")
    w2_d = nc.dram_tensor("w2", [128, HID // 128, Z_DIM], bf16,
                          kind="ExternalInput")
    redtab_d = nc.dram_tensor("redtab", [Z_DIM, NUM_STEPS + 1], f32r,
                              kind="ExternalInput")
    b1t_d = nc.dram_tensor("b1t", [128, HID // 128, NUM_STEPS], f32,
                           kind="ExternalInput")
    b2t_d = nc.dram_tensor("b2t", [Z_DIM, NUM_STEPS], f32,
                           kind="ExternalInput")
    muneg_d = nc.dram_tensor("muneg", [Z_DIM, 1], f32, kind="ExternalInput")

    xt_d = nc.dram_tensor("xt", [NUM_STEPS + 1, NCH, Z_DIM, CW], f32,
                          kind="ExternalOutput")
    lw_d = nc.dram_tensor("lw", [NCH, 1, CW], f32, kind="ExternalOutput")
    nab_d = nc.dram_tensor("nab", [NCH, Z_DIM, CW], f32, kind="ExternalOutput")

    MT = HID // 128  # 4 m-tiles

    with tile.TileContext(nc) as tc:
        with (
            tc.tile_pool(name="const", bufs=1) as cpool,
            tc.tile_pool(name="state", bufs=1) as spool,
            tc.tile_pool(name="noise", bufs=6) as npool,
            tc.tile_pool(name="work", bufs=3) as wpool,
            tc.tile_pool(name="hbuf", bufs=2) as hpool,
            tc.tile_pool(name="hps", bufs=4, space="PSUM") as hps_pool,
            tc.tile_pool(name="sps", bufs=2, space="PSUM") as sps_pool,
            tc.tile_pool(name="lwp", bufs=1, space="PSUM") as lw_pool,
        ):
            # --- constants / weights ---
            w1a = cpool.tile([KT1, HID], bf16)
            nc.sync.dma_start(out=w1a[:], in_=w1_d[0:KT1, :])
            w1b = cpool.tile([KT2, HID], bf16)
            nc.sync.dma_start(out=w1b[:], in_=w1_d[KT1:KT1 + KT2, :])
            w2 = cpool.tile([128, MT, Z_DIM], bf16)
            nc.sync.dma_start(out=w2[:], in_=w2_d[:])
            b1t = cpool.tile([128, MT, NUM_STEPS], f32)
            nc.sync.dma_start(out=b1t[:], in_=b1t_d[:])
            b2t = cpool.tile([Z_DIM, NUM_STEPS], f32)
            nc.sync.dma_start(out=b2t[:], in_=b2t_d[:])
            muneg = cpool.tile([Z_DIM, 1], f32)
            nc.sync.dma_start(out=muneg[:], in_=muneg_d[:])
            redtab = cpool.tile([Z_DIM, NUM_STEPS + 1], f32r)
            nc.sync.dma_start(out=redtab[:], in_=redtab_d[:])

            # --- per-chunk state ---
            # inA[parity][c]: [128, CW]; rows 0:64 = x (rewritten every step),
            # rows 64:128 = r[0:64] (constant). inB[c]: [64, CW] = r[64:128].
            inA = [[spool.tile([128, CW], bf16, name=f"inA{p}c{c}")
                    for c in range(NCH)] for p in range(2)]
            xst = [[spool.tile([Z_DIM, CW], f32, name=f"xst{p}c{c}")
                    for c in range(NCH)] for p in range(2)]
            inB = []
            for c in range(NCH):
                for p in range(2):
                    nc.sync.dma_start(out=inA[p][c][Z_DIM:128, :],
                                      in_=r_d[c, 0:64, :])
                t = spool.tile([KT2, CW], bf16, name=f"inB{c}")
                nc.sync.dma_start(out=t[:], in_=r_d[c, 64:128, :])
                inB.append(t)
                nc.sync.dma_start(out=xst[0][c][:], in_=x0_d[c])
                nc.vector.tensor_copy(inA[0][c][0:Z_DIM, :], xst[0][c][:])
                # trajectory slab 0 = x0
                nc.sync.dma_start(out=xt_d[0, c], in_=xst[0][c][:])

            lw_ps = [lw_pool.tile([1, CW], f32, name=f"lwps{c}")
                     for c in range(NCH)]

            # --- main loop ---
            for s in range(NUM_STEPS):
                cs = float(1.0 - np.sqrt(1.0 - alpha[s]))
                sa = float(ETA * np.sqrt(alpha[s]))
                cs1 = float(np.float32(1.0) - np.float32(cs))
                twosa = float(2.0 * np.sqrt(alpha[s]))
                nka = float(-1.0 / (2.0 * alpha[s]))
                cur, nxt = inA[s % 2], inA[(s + 1) % 2]
                xcur, xnxt = xst[s % 2], xst[(s + 1) % 2]
                for c in range(NCH):
                    n_t = npool.tile([Z_DIM, CW], f32, tag="noise")
                    nc.sync.dma_start(out=n_t[:], in_=noises_d[s, c])

                    # MM1: h_pre = [x; r].T-major matmul, K = 128 + 64
                    h = hpool.tile([128, MT, CW], bf16, tag=f"h{c}")
                    for m in range(MT):
                        ps = hps_pool.tile([128, CW], f32, tag="hps")
                        nc.tensor.matmul(ps[:],
                                         w1a[:, m * 128:(m + 1) * 128],
                                         cur[c][:], start=True, stop=False)
                        nc.tensor.matmul(ps[:],
                                         w1b[:, m * 128:(m + 1) * 128],
                                         inB[c][:], start=False, stop=True)
                        # tanh(h_pre + b1 + t_s * W1[192]) fused via bias table
                        nc.scalar.activation(h[:, m, :], ps[:], Act.Tanh,
                                             bias=b1t[:, m, s:s + 1], scale=1.0)

                    # MM2: score = h @ W2 (K = 512 in 4 tiles)
                    sps = sps_pool.tile([Z_DIM, CW], f32, tag="sps")
                    for k in range(MT):
                        nc.tensor.matmul(sps[:], w2[:, k, :], h[:, k, :],
                                         start=(k == 0), stop=(k == MT - 1))
                    # sc2 = 2*cs*(score + b2), via scale/bias fold on ScalarE
                    sc2 = wpool.tile([Z_DIM, CW], f32, tag="sc")
                    nc.scalar.activation(sc2[:], sps[:], Act.Identity,
                                         bias=b2t[:, s:s + 1], scale=2.0 * cs)

                    x = xcur[c][:]
                    # log-weight increment, accumulated in PSUM via matmul:
                    #   -2ka*s^2 - 2sqrt(ka)*s*n == -1/(2a) * sc2*(sc2+2sqrt(a)n)
                    # the -1/(2a) lives in redtab[:, s].
                    w_t = wpool.tile([Z_DIM, CW], f32, tag="w")
                    nc.vector.scalar_tensor_tensor(w_t[:], n_t[:], twosa,
                                                   sc2[:], Alu.mult, Alu.add)
                    p_t = wpool.tile([Z_DIM, CW], f32r, tag="p")
                    nc.vector.tensor_tensor(p_t[:], sc2[:], w_t[:], Alu.mult)
                    nc.tensor.matmul(lw_ps[c][:], redtab[:, s:s + 1], p_t[:],
                                     start=(s == 0), stop=False)

                    # x_new = (1-cs)*x + sc2 + sa*n
                    m_t = wpool.tile([Z_DIM, CW], f32, tag="m")
                    nc.vector.scalar_tensor_tensor(m_t[:], x, cs1, sc2[:],
                                                   Alu.mult, Alu.add)
                    xn = xnxt[c][:]
                    nc.vector.scalar_tensor_tensor(xn, n_t[:], sa, m_t[:],
                                                   Alu.mult, Alu.add)
                    nc.vector.tensor_copy(nxt[c][0:Z_DIM, :], xn)
                    nc.sync.dma_start(out=xt_d[s + 1, c], in_=xn)

            # --- epilogue: log_weights and nabla_g ---
            xfin = xst[NUM_STEPS % 2]
            for c in range(NCH):
                x = xfin[c][:]
                # terminal log-weight 0.5*(x^2 - (x-mu)^2): redtab[:, S] = 0.5
                xm = wpool.tile([Z_DIM, CW], f32, tag="w")
                nc.scalar.activation(xm[:], x, Act.Square,
                                     bias=muneg[:, 0:1], scale=1.0)
                xsq = wpool.tile([Z_DIM, CW], f32, tag="m")
                nc.scalar.activation(xsq[:], x, Act.Square, bias=0.0, scale=1.0)
                td = wpool.tile([Z_DIM, CW], f32r, tag="p")
                nc.vector.tensor_tensor(td[:], xsq[:], xm[:], Alu.subtract)
                nc.tensor.matmul(lw_ps[c][:],
                                 redtab[:, NUM_STEPS:NUM_STEPS + 1], td[:],
                                 start=False, stop=True)
                lw_sb = wpool.tile([1, CW], f32, tag="lwsb")
                nc.vector.tensor_copy(lw_sb[:], lw_ps[c][:])
                nc.sync.dma_start(out=lw_d[c], in_=lw_sb[:])

                # nabla_g = (x - mu) - x  (== -mu up to rounding, as reference)
                d1 = wpool.tile([Z_DIM, CW], f32, tag="m")
                nc.scalar.activation(d1[:], x, Act.Identity,
                                     bias=muneg[:, 0:1], scale=1.0)
                nab = wpool.tile([Z_DIM, CW], f32, tag="sc")
                nc.vector.tensor_tensor(nab[:], d1[:], x, Alu.subtract)
                nc.sync.dma_start(out=nab_d[c], in_=nab[:])

    nc.compile()
    return nc


def kernel(r, noises, x0, W1, b1, W2, b2, beta_max, beta_min, target_mu):
    from concourse.bass_utils import run_bass_kernel_spmd

    r = np.asarray(r, np.float32)
    noises = np.asarray(noises, np.float32)
    x0 = np.asarray(x0, np.float32)
    W1 = np.asarray(W1, np.float32)
    b1 = np.asarray(b1, np.float32)
    W2 = np.asarray(W2, np.float32)
    b2 = np.asarray(b2, np.float32)
    bmax = float(np.asarray(beta_max))
    bmin = float(np.asarray(beta_min))
    mu = np.asarray(target_mu, np.float32)

    ts, alpha, kappa = _schedule(bmax, bmin)

    key = (bmax, bmin)
    if key not in _CACHE:
        _CACHE[key] = _build(ts, alpha, kappa)
    nc = _CACHE[key]

    # Host-side shard prep (pure data movement + the schedule-bias fold).
    r_flat = r.reshape(B, R_DIM)
    # bias table: b1 + t_s * W1[last row] -> [128, 4, S]
    b1t = (b1[None, :] + ts[:, None] * W1[Z_DIM + R_DIM, :]).astype(np.float32)
    b1t = np.ascontiguousarray(
        b1t.reshape(NUM_STEPS, HID // 128, 128).transpose(2, 1, 0))
    import ml_dtypes
    bf = ml_dtypes.bfloat16
    w1_in = np.ascontiguousarray(W1[: Z_DIM + R_DIM]).astype(bf)
    w2_in = np.ascontiguousarray(
        W2.reshape(HID // 128, 128, Z_DIM).transpose(1, 0, 2)).astype(bf)
    redtab_in = np.empty((Z_DIM, NUM_STEPS + 1), np.float32)
    redtab_in[:, :NUM_STEPS] = (-1.0 / (2.0 * alpha))[None, :]
    redtab_in[:, NUM_STEPS] = 0.5
    cs_all = (1.0 - np.sqrt(1.0 - alpha)).astype(np.float32)  # [S]
    b2t_in = np.ascontiguousarray(
        (2.0 * cs_all[None, :] * b2[:, None]).astype(np.float32))
    muneg_in = np.ascontiguousarray(-mu[:, None])

    in_maps = []
    for i in range(N_CORES):
        b0, b1i = i * BS, (i + 1) * BS
        nsh = noises[b0:b1i]            # [BS, S, D]
        nT = np.ascontiguousarray(
            nsh.reshape(NCH, CW, NUM_STEPS, Z_DIM).transpose(2, 0, 3, 1))
        x0T = np.ascontiguousarray(
            x0[b0:b1i].reshape(NCH, CW, Z_DIM).transpose(0, 2, 1))
        rT = np.ascontiguousarray(
            r_flat[b0:b1i].reshape(NCH, CW, R_DIM).transpose(0, 2, 1)
        ).astype(bf)
        in_maps.append({
            "noises": nT, "x0": x0T, "r": rT,
            "w1": w1_in, "w2": w2_in, "b1t": b1t,
            "b2t": b2t_in, "muneg": muneg_in, "redtab": redtab_in,
        })

    res = run_bass_kernel_spmd(nc, in_maps, core_ids=list(range(N_CORES)),
                               trace=TRACE)
    _LAST_RESULT.clear()
    _LAST_RESULT["exec_time_ns"] = res.exec_time_ns
    _LAST_RESULT["trace"] = (res.instructions_and_trace[1]
                             if res.instructions_and_trace else None)

    x_t = np.empty((B, NUM_STEPS + 1, Z_DIM), np.float32)
    log_w = np.empty((B,), np.float32)
    nab = np.empty((B, Z_DIM), np.float32)
    for i in range(N_CORES):
        b0, b1i = i * BS, (i + 1) * BS
        out = res.results[i]
        # [S+1, NCH, D, CW] -> [NCH, CW, S+1, D]
        x_t[b0:b1i] = out["xt"].transpose(1, 3, 0, 2).reshape(
            BS, NUM_STEPS + 1, Z_DIM)
        log_w[b0:b1i] = out["lw"].reshape(BS)
        nab[b0:b1i] = out["nab"].transpose(0, 2, 1).reshape(BS, Z_DIM)
    return x_t, log_w, nab
